# revision 38
# baseline (speedup 1.0000x reference)
"""Trainium2 Bass kernel for grouped vector attention (sparse_attention).

Reference computation (B=2, L1=L2=512, D=256, g=16, n=16):
    Q = x_target @ Wq.T ; K = x_source @ Wk.T ; V = x_source @ Wv.T
    diff = Q.reshape(B,L1,1,n,g) - K.reshape(B,1,L2,n,g)
    scores = relu(einsum('bijng,g->bijn', relu(diff), w_mlp) + b_mlp)
    att = softmax(scores, axis=2)                      # over L2
    out = einsum('bijn,bjgn->bign', att, V.reshape(B,L2,g,n)).reshape(B,L1,D)

Sharding: 8 cores = 2 batches x 4 L2(j)-quarters. Each core handles all 512
queries against its 128 source positions and produces partial (unnormalized)
outputs + partial softmax denominators; the host sums the 4 partials per
batch and divides. Sharding over j (not i) means the exp'd scores come out
with j on partitions — exactly what the att@V contraction needs, so there is
no on-chip transpose anywhere.

Per-core pipeline, for each quad of 4 source positions (32 quads):
  - tmp[d, i] = relu(Q[i,d] - K[j,d]) with d on partitions, i free:
      ScalarE:  activation(Relu, in=QT, bias=-K[:,j], scale=1)
      VectorE:  tensor_scalar(in=QT, s1=-K[:,j], s2=0, op0=add, op1=max)
  - grouped weighted sum over g=16 via TensorE matmul with block-diagonal
    [128 x 32] sel (w_mlp folded); j's 16 scores land in PSUM slot 32*jj.
  - p = exp(scores + b) off PSUM; pc = max(p, 1)   (= exp(relu(scores+b)))
  - V_sel[g][32*jj + nn, e] = V[4g+jj, e] * (e % 16 == nn)  (built once by a
    broadcast-DMA from a DRAM copy of V + one masked multiply)
  - out_partial[e, i]  += V_sel[g][:, e-half].T @ pc   (PSUM accumulation
    across all 32 quads);  S_partial[nn, i] += ones_sel.T @ pc
"""

import numpy as np

import concourse.bass as bass
import concourse.bacc as bacc
import concourse.tile as tile
import concourse.mybir as mybir
from concourse.bass_utils import run_bass_kernel_spmd

import ml_dtypes

F32 = mybir.dt.float32
BF16 = mybir.dt.bfloat16
AL = mybir.AluOpType
AF = mybir.ActivationFunctionType

B, L1, L2, D = 2, 512, 512, 256
G = 16           # group size (d_group)
N = 16           # number of groups
NCORES = 8
JSH = 128        # source positions per core (L2 / 4)
NQUAD = 32       # 32 quads of 4 source positions
BF = ml_dtypes.bfloat16

# elementwise engine rotation per (j,h) unit: 0=VectorE, 1=ScalarE, 2=GpSimd
# (GpSimd tensor_scalar measured 7.5us/op on HW - never assign it)
ENGINE_PATTERN = (
    0, 1, 0, 0, 1, 0, 1, 0, 0, 1, 0, 0, 1, 0, 1, 0,
    0, 1, 0, 0, 1, 0, 1, 0, 0, 1, 0, 0, 1, 0, 0, 0,
)
GPS_TT = False  # GpSimd streaming degrades VectorE via the shared SBUF port


def _build(b_val: float):
    """Build + compile the per-core Bass graph. Same graph for all 8 cores."""
    nc = bacc.Bacc(
        "TRN2", target_bir_lowering=False, debug=False, enable_asserts=False
    )

    # ---- DRAM parameters (per-core shards, host-prepped) ----
    xtT_d = nc.dram_tensor("xtT", [2, 128, L1], BF16, kind="ExternalInput")
    xssT_d = nc.dram_tensor("xssT", [2, 128, JSH], BF16, kind="ExternalInput")
    wqT_d = nc.dram_tensor("wqT", [2, 128, D], BF16, kind="ExternalInput")
    wkT_d = nc.dram_tensor("wkT", [2, 128, D], BF16, kind="ExternalInput")
    wvT_d = nc.dram_tensor("wvT", [2, 128, D], BF16, kind="ExternalInput")
    sel_d = nc.dram_tensor("sel", [2, 128, 32], BF16, kind="ExternalInput")
    vmask_d = nc.dram_tensor("vmask", [128, 2 * D], BF16, kind="ExternalInput")
    ones_d = nc.dram_tensor("ones_sel", [128, N], BF16, kind="ExternalInput")
    outp_d = nc.dram_tensor("outp", [2, 128, L1], F32, kind="ExternalOutput")
    souts_d = nc.dram_tensor("souts", [N, L1], F32, kind="ExternalOutput")
    vdram = nc.dram_tensor("vdram", [JSH, D], BF16)

    with tile.TileContext(nc) as tc:
        with (
            tc.tile_pool(name="const", bufs=1) as cpool,
            tc.tile_pool(name="vselp", bufs=1) as vpool,
            tc.tile_pool(name="work", bufs=4) as wpool,
            tc.tile_pool(name="tmps", bufs=9) as tpool,
            tc.tile_pool(name="ps_s", bufs=2, space="PSUM") as ps_pool,
            tc.tile_pool(name="ps_acc", bufs=1, space="PSUM") as pa_pool,
        ):
            # ---- load constants / inputs ----
            xtT = [cpool.tile([128, L1], BF16, name=f"xtT{h}") for h in range(2)]
            xssT = [cpool.tile([128, JSH], BF16, name=f"xssT{h}") for h in range(2)]
            wqT = [cpool.tile([128, D], BF16, name=f"wqT{h}") for h in range(2)]
            wkT = [cpool.tile([128, D], BF16, name=f"wkT{h}") for h in range(2)]
            wvT = [cpool.tile([128, D], BF16, name=f"wvT{h}") for h in range(2)]
            sel = [cpool.tile([128, 32], BF16, name=f"sel{h}") for h in range(2)]
            vmask = cpool.tile([128, 2 * D], BF16, name="vmask")
            ones_sel = cpool.tile([128, N], BF16, name="ones_sel")
            bml = cpool.tile([128, 1], F32, name="bml")
            nc.vector.memset(bml[:], float(b_val))

            # ---- accumulators (also the warm-up target: quad 0's V-matmul
            # uses start=True, which clears whatever the warm-up wrote) ----
            ops = [
                pa_pool.tile([128, L1], F32, name=f"ops{eh}") for eh in range(2)
            ]
            sps = pa_pool.tile([16, L1], F32, name="sps")

            # ---- PE warm-up burst: self-contained (memset inputs), runs at
            # t~0 so HAM flips to 8/8 and stays there until real matmuls flow
            wz = cpool.tile([128, L1], BF16, name="wz")
            nc.vector.memset(wz[:], 0.25)
            for k in range(20):
                nc.tensor.matmul(
                    ops[0][0:32, 0:256],
                    wz[:, 0:32],
                    wz[:, 0:256],
                    start=(k == 0),
                    stop=(k == 19),
                    skip_group_check=True,
                )
            for h in range(2):
                nc.sync.dma_start(xssT[h][:], xssT_d[h])
                nc.sync.dma_start(wvT[h][:], wvT_d[h])
                nc.sync.dma_start(sel[h][:], sel_d[h])
                nc.sync.dma_start(xtT[h][:], xtT_d[h])
                nc.scalar.dma_start(wqT[h][:], wqT_d[h])
                nc.scalar.dma_start(wkT[h][:], wkT_d[h])
            nc.scalar.dma_start(vmask[:], vmask_d[:])
            nc.scalar.dma_start(ones_sel[:], ones_d[:])

            # ---- V first: projection -> DRAM -> per-quad broadcast+mask ----
            Vt = cpool.tile([128, D], BF16, name="Vt")
            psv = ps_pool.tile([128, D], F32, name="psv", tag="psk", bufs=1)
            for dh in range(2):
                nc.tensor.matmul(
                    psv[:],
                    xssT[dh][:],
                    wvT[dh][:],
                    start=(dh == 0),
                    stop=(dh == 1),
                )
            nc.vector.tensor_copy(Vt[:], psv[:])
            nc.sync.dma_start(vdram[:], Vt[:])

            V_sel = []
            for k in range(NQUAD // 2):
                vs2 = vpool.tile([128, 2 * D], BF16, name=f"vs{k}")
                for half in range(2):
                    g = 2 * k + half
                    bsrc = (
                        vdram.ap()[4 * g : 4 * g + 4, :]
                        .unsqueeze(1)
                        .broadcast_to((4, 32, D))
                    )
                    nc.sync.dma_start(vs2[:, half * D : (half + 1) * D], bsrc)
                nc.vector.tensor_tensor(vs2[:], vs2[:], vmask[:], op=AL.mult)
                V_sel.append(vs2)


            # ---- projections on device ----
            # QT[h] (128 e, 512 i) bf16
            QT = [cpool.tile([128, L1], BF16, name=f"QT{h}") for h in range(2)]
            for eh in range(2):
                psq = ps_pool.tile([128, 2 * L1], F32, name="psq", tag="ps_s")
                for dh in range(2):
                    nc.tensor.matmul(
                        psq[:, 0:L1],
                        wqT[dh][:, eh * 128 : (eh + 1) * 128],
                        xtT[dh][:],
                        start=(dh == 0),
                        stop=(dh == 1),
                    )
                nc.vector.tensor_copy(QT[eh][:], psq[:, 0:L1])
            # KTn[h] (128 e, 128 j) f32, negated (bias / scalar operand)
            KTn = [cpool.tile([128, JSH], F32, name=f"KTn{h}") for h in range(2)]
            for eh in range(2):
                psk = ps_pool.tile([128, JSH], F32, name="psk", tag="psk", bufs=1)
                for dh in range(2):
                    nc.tensor.matmul(
                        psk[:],
                        wkT[dh][:, eh * 128 : (eh + 1) * 128],
                        xssT[dh][:],
                        start=(dh == 0),
                        stop=(dh == 1),
                    )
                nc.vector.tensor_scalar(KTn[eh][:], psk[:], -1.0, None, AL.mult)


            # PE filler target (keeps HAM warm through the ramp; reuses the
            # prep psum slot, never read)
    
            wu2 = ps_pool.tile([32, 256], F32, name="wu2", tag="psk", bufs=1)

            def pe_fill(n):
                for _ in range(n):
                    nc.tensor.matmul(
                        wu2[:],
                        wz[:, 0:32],
                        wz[:, 0:256],
                        start=True,
                        stop=True,
                        skip_group_check=True,
                    )

            # ---- main loop: 16 double-quads of 8 source positions ----
            for dq in range(NQUAD // 2):
                ps = ps_pool.tile([128, 2 * L1], F32, name="ps", tag="ps_s")
                for half2 in range(2):
                    g = 2 * dq + half2
                    for jj in range(4):
                        j = 4 * g + jj
                        for h in range(2):
                            u = (g * 4 + jj) * 2 + h
                            eng = ENGINE_PATTERN[u % len(ENGINE_PATTERN)]
                            t = tpool.tile([128, L1], BF16, name="t", tag="t")
                            if eng == 1:
                                # t = relu(Q + (-K))
                                nc.scalar.activation(
                                    t[:],
                                    QT[h][:],
                                    AF.Relu,
                                    bias=KTn[h][:, j : j + 1],
                                    scale=1.0,
                                )
                            else:
                                # t = max(Q + (-K), 0)
                                nc.vector.tensor_scalar(
                                    t[:],
                                    QT[h][:],
                                    KTn[h][:, j : j + 1],
                                    0.0,
                                    AL.add,
                                    AL.max,
                                )
                            nc.tensor.matmul(
                                ps[
                                    32 * jj : 32 * jj + 32,
                                    L1 * half2 : L1 * half2 + L1,
                                ],
                                sel[h][:],
                                t[:],
                                start=(h == 0),
                                stop=(h == 1),
                                tile_position=(0, 32 * jj),
                            )
                # p = exp(scores + b); pc = max(p, 1) = exp(relu(scores + b))
                p = wpool.tile([128, 2 * L1], BF16, name="p", tag="p", bufs=4)
                nc.scalar.activation(p[:], ps[:], AF.Exp, bias=bml[:], scale=1.0)
                pc = wpool.tile([128, 2 * L1], BF16, name="pc", tag="pc", bufs=8)
                nc.vector.tensor_scalar(pc[:], p[:], 1.0, None, AL.max)
                # accumulate partial outputs and denominators
                for half2 in range(2):
                    g = 2 * dq + half2
                    pcs = pc[:, L1 * half2 : L1 * half2 + L1]
                    for eh in range(2):
                        off = half2 * D + eh * 128
                        nc.tensor.matmul(
                            ops[eh][:],
                            V_sel[dq][:, off : off + 128],
                            pcs,
                            start=(g == 0),
                            stop=(g == NQUAD - 1),
                            skip_group_check=True,
                        )
                    nc.tensor.matmul(
                        sps[:],
                        ones_sel[:, 0:N],
                        pcs,
                        start=(g == 0),
                        stop=(g == NQUAD - 1),
                        skip_group_check=True,
                    )

            # ---- evacuate + store ----
            for eh in range(2):
                ou = wpool.tile([128, L1], F32, name="ou", tag="ou", bufs=2)
                ceng = nc.vector if eh == 0 else nc.scalar
                if eh == 0:
                    nc.vector.tensor_copy(ou[:], ops[eh][:])
                else:
                    nc.scalar.copy(ou[:], ops[eh][:])
                nc.sync.dma_start(outp_d[eh], ou[:])
            so = wpool.tile([16, L1], F32, name="so")
            nc.scalar.copy(so[:], sps[:])
            nc.sync.dma_start(souts_d[:], so[:])

    nc.compile()
    return nc


_CACHE: dict = {}


def _get_graph(b_val: float):
    key = round(float(b_val), 10)
    if key not in _CACHE:
        _CACHE[key] = _build(float(b_val))
    return _CACHE[key]


def _host_prep(x_source, x_target, Wq, Wk, Wv, w_mlp):
    """Build per-core input maps (numpy, bf16)."""
    w_full = np.tile(np.asarray(w_mlp, np.float32), D // G)  # w_full[d] = w[d%16]
    sel = np.zeros((2, 128, 32), np.float32)
    for h in range(2):
        for dl in range(128):
            d = 128 * h + dl
            sel[h, dl, d // G] = w_full[d]
    # V_sel mask: row p = 32*jj + s (s<16 valid), col e: keep if e%16 == s
    vmask = np.zeros((128, 2 * D), np.float32)
    for p in range(128):
        s = p % 32
        if s < 16:
            vmask[p, s::G] = 1.0
    # S selector: row p = 32*jj + s -> column s (s < 16)
    ones_sel = np.zeros((128, N), np.float32)
    for p in range(128):
        s = p % 32
        if s < 16:
            ones_sel[p, s] = 1.0

    def split_h(a):  # (256, X) -> (2, 128, X)
        return np.ascontiguousarray(a.reshape(2, 128, a.shape[1]))

    wq_b = split_h(np.asarray(Wq, np.float32).T).astype(BF)
    wk_b = split_h(np.asarray(Wk, np.float32).T).astype(BF)
    wv_b = split_h(np.asarray(Wv, np.float32).T).astype(BF)
    sel_b = sel.astype(BF)
    vmask_b = vmask.astype(BF)
    ones_b = ones_sel.astype(BF)

    xtT = [
        split_h(np.asarray(x_target[b], np.float32).T).astype(BF) for b in range(B)
    ]
    xsT = [np.asarray(x_source[b], np.float32).T for b in range(B)]
    in_maps = []
    for core in range(NCORES):
        b, jq = divmod(core, 4)
        j0 = jq * JSH
        xssT = split_h(xsT[b][:, j0 : j0 + JSH]).astype(BF)
        in_maps.append(
            {
                "xtT": xtT[b],
                "xssT": xssT,
                "wqT": wq_b,
                "wkT": wk_b,
                "wvT": wv_b,
                "sel": sel_b,
                "vmask": vmask_b,
                "ones_sel": ones_b,
            }
        )
    return in_maps


def _host_gather(results):
    """Sum partials over j-shards, normalize, reshape to (B, L1, D)."""
    out = np.empty((B, L1, D), np.float32)
    for b in range(B):
        cores = [b * 4 + jq for jq in range(4)]
        U = sum(
            results[c]["outp"].reshape(D, L1).astype(np.float64) for c in cores
        )  # (e, i)
        S = sum(results[c]["souts"].astype(np.float64) for c in cores)  # (nn, i)
        att = U / S[np.arange(D) % N, :]  # (e, i)
        out[b] = att.T.astype(np.float32)
    return out


def run(inputs, trace=False, **kwargs):
    nc = _get_graph(float(np.asarray(inputs["b_mlp"]).reshape(-1)[0]))
    in_maps = _host_prep(
        inputs["x_source"],
        inputs["x_target"],
        inputs["Wq"],
        inputs["Wk"],
        inputs["Wv"],
        inputs["w_mlp"],
    )
    res = run_bass_kernel_spmd(
        nc, in_maps, core_ids=list(range(NCORES)), trace=trace, **kwargs
    )
    return _host_gather(res.results), res


def kernel(**inputs) -> np.ndarray:
    out, _ = run(inputs, trace=False)
    return out


# revision 40
# speedup vs baseline: 1.0646x; 1.0646x over previous
"""Trainium2 Bass kernel for grouped vector attention (sparse_attention).

Reference computation (B=2, L1=L2=512, D=256, g=16, n=16):
    Q = x_target @ Wq.T ; K = x_source @ Wk.T ; V = x_source @ Wv.T
    diff = Q.reshape(B,L1,1,n,g) - K.reshape(B,1,L2,n,g)
    scores = relu(einsum('bijng,g->bijn', relu(diff), w_mlp) + b_mlp)
    att = softmax(scores, axis=2)                      # over L2
    out = einsum('bijn,bjgn->bign', att, V.reshape(B,L2,g,n)).reshape(B,L1,D)

Sharding: 8 cores = 2 batches x 4 L2(j)-quarters. Each core handles all 512
queries against its 128 source positions and produces partial (unnormalized)
outputs + partial softmax denominators; the host sums the 4 partials per
batch and divides. Sharding over j (not i) means the exp'd scores come out
with j on partitions — exactly what the att@V contraction needs, so there is
no on-chip transpose anywhere.

Per-core pipeline, for each quad of 4 source positions (32 quads):
  - tmp[d, i] = relu(Q[i,d] - K[j,d]) with d on partitions, i free:
      ScalarE:  activation(Relu, in=QT, bias=-K[:,j], scale=1)
      VectorE:  tensor_scalar(in=QT, s1=-K[:,j], s2=0, op0=add, op1=max)
  - grouped weighted sum over g=16 via TensorE matmul with block-diagonal
    [128 x 32] sel (w_mlp folded); j's 16 scores land in PSUM slot 32*jj.
  - p = exp(scores + b) off PSUM; pc = max(p, 1)   (= exp(relu(scores+b)))
  - V_sel[g][32*jj + nn, e] = V[4g+jj, e] * (e % 16 == nn)  (built once by a
    broadcast-DMA from a DRAM copy of V + one masked multiply)
  - out_partial[e, i]  += V_sel[g][:, e-half].T @ pc   (PSUM accumulation
    across all 32 quads);  S_partial[nn, i] += ones_sel.T @ pc
"""

import numpy as np

import concourse.bass as bass
import concourse.bacc as bacc
import concourse.tile as tile
import concourse.mybir as mybir
from concourse.bass_utils import run_bass_kernel_spmd

import ml_dtypes

F32 = mybir.dt.float32
BF16 = mybir.dt.bfloat16
AL = mybir.AluOpType
AF = mybir.ActivationFunctionType

B, L1, L2, D = 2, 512, 512, 256
G = 16           # group size (d_group)
N = 16           # number of groups
NCORES = 8
JSH = 128        # source positions per core (L2 / 4)
NQUAD = 32       # 32 quads of 4 source positions
BF = ml_dtypes.bfloat16

# elementwise engine rotation per (j,h) unit: 0=VectorE, 1=ScalarE, 2=GpSimd
# (GpSimd tensor_scalar measured 7.5us/op on HW - never assign it)
ENGINE_PATTERN = (
    0, 1, 0, 0, 1, 0, 1, 0, 0, 1, 0, 0, 1, 0, 1, 0,
    0, 1, 0, 0, 1, 0, 1, 0, 0, 1, 0, 0, 1, 0, 0, 0,
)
GPS_TT = False  # GpSimd streaming degrades VectorE via the shared SBUF port


def _build(b_val: float):
    """Build + compile the per-core Bass graph. Same graph for all 8 cores."""
    nc = bacc.Bacc(
        "TRN2", target_bir_lowering=False, debug=False, enable_asserts=False
    )

    # ---- DRAM parameters (per-core shards, host-prepped) ----
    xtT_d = nc.dram_tensor("xtT", [2, 128, L1], BF16, kind="ExternalInput")
    xssT_d = nc.dram_tensor("xssT", [2, 128, JSH], BF16, kind="ExternalInput")
    wqT_d = nc.dram_tensor("wqT", [2, 128, D], BF16, kind="ExternalInput")
    wkT_d = nc.dram_tensor("wkT", [2, 128, D], BF16, kind="ExternalInput")
    wvT_d = nc.dram_tensor("wvT", [2, 128, D], BF16, kind="ExternalInput")
    sel_d = nc.dram_tensor("sel", [2, 128, 32], BF16, kind="ExternalInput")
    vmask_d = nc.dram_tensor("vmask", [128, 2 * D], BF16, kind="ExternalInput")
    ones_d = nc.dram_tensor("ones_sel", [128, N], BF16, kind="ExternalInput")
    outp_d = nc.dram_tensor("outp", [2, 128, L1], F32, kind="ExternalOutput")
    souts_d = nc.dram_tensor("souts", [N, L1], F32, kind="ExternalOutput")
    vdram = nc.dram_tensor("vdram", [JSH, D], BF16)

    with tile.TileContext(nc) as tc:
        with (
            tc.tile_pool(name="const", bufs=1) as cpool,
            tc.tile_pool(name="vselp", bufs=1) as vpool,
            tc.tile_pool(name="work", bufs=4) as wpool,
            tc.tile_pool(name="tmps", bufs=9) as tpool,
            tc.tile_pool(name="ps_s", bufs=3, space="PSUM") as ps_pool,
            tc.tile_pool(name="ps_acc", bufs=1, space="PSUM") as pa_pool,
        ):
            # ---- load constants / inputs ----
            xtT = [cpool.tile([128, L1], BF16, name=f"xtT{h}") for h in range(2)]
            xssT = [cpool.tile([128, JSH], BF16, name=f"xssT{h}") for h in range(2)]
            wqT = [cpool.tile([128, D], BF16, name=f"wqT{h}") for h in range(2)]
            wkT = [cpool.tile([128, D], BF16, name=f"wkT{h}") for h in range(2)]
            wvT = [cpool.tile([128, D], BF16, name=f"wvT{h}") for h in range(2)]
            sel = [cpool.tile([128, 32], BF16, name=f"sel{h}") for h in range(2)]
            vmask = cpool.tile([128, 2 * D], BF16, name="vmask")
            ones_sel = cpool.tile([128, N], BF16, name="ones_sel")
            bml = cpool.tile([128, 1], F32, name="bml")
            nc.vector.memset(bml[:], float(b_val))

            # ---- accumulators (also the warm-up target: quad 0's V-matmul
            # uses start=True, which clears whatever the warm-up wrote) ----
            ops = [
                pa_pool.tile([128, L1], F32, name=f"ops{eh}") for eh in range(2)
            ]
            sps = pa_pool.tile([16, L1], F32, name="sps")

            # ---- PE warm-up burst: self-contained (memset inputs), runs at
            # t~0 so HAM flips to 8/8 and stays there until real matmuls flow
            wz = cpool.tile([128, L1], BF16, name="wz")
            nc.vector.memset(wz[:], 0.25)
            for k in range(20):
                nc.tensor.matmul(
                    ops[0][0:32, 0:256],
                    wz[:, 0:32],
                    wz[:, 0:256],
                    start=(k == 0),
                    stop=(k == 19),
                    skip_group_check=True,
                )
            for h in range(2):
                nc.sync.dma_start(xssT[h][:], xssT_d[h])
                nc.sync.dma_start(wvT[h][:], wvT_d[h])
                nc.sync.dma_start(sel[h][:], sel_d[h])
                nc.sync.dma_start(xtT[h][:], xtT_d[h])
                nc.scalar.dma_start(wqT[h][:], wqT_d[h])
                nc.scalar.dma_start(wkT[h][:], wkT_d[h])
            nc.scalar.dma_start(vmask[:], vmask_d[:])
            nc.scalar.dma_start(ones_sel[:], ones_d[:])

            # ---- V first: projection -> DRAM -> per-quad broadcast+mask ----
            Vt = cpool.tile([128, D], BF16, name="Vt")
            psv = ps_pool.tile([128, D], F32, name="psv", tag="psk", bufs=1)
            for dh in range(2):
                nc.tensor.matmul(
                    psv[:],
                    xssT[dh][:],
                    wvT[dh][:],
                    start=(dh == 0),
                    stop=(dh == 1),
                )
            nc.vector.tensor_copy(Vt[:], psv[:])
            nc.sync.dma_start(vdram[:], Vt[:])

            V_sel = []
            for k in range(NQUAD // 2):
                vs2 = vpool.tile([128, 2 * D], BF16, name=f"vs{k}")
                for half in range(2):
                    g = 2 * k + half
                    bsrc = (
                        vdram.ap()[4 * g : 4 * g + 4, :]
                        .unsqueeze(1)
                        .broadcast_to((4, 32, D))
                    )
                    nc.sync.dma_start(vs2[:, half * D : (half + 1) * D], bsrc)
                nc.vector.tensor_tensor(vs2[:], vs2[:], vmask[:], op=AL.mult)
                V_sel.append(vs2)


            # ---- projections on device ----
            # QT[h] (128 e, 512 i) bf16
            QT = [cpool.tile([128, L1], BF16, name=f"QT{h}") for h in range(2)]
            for eh in range(2):
                psq = ps_pool.tile([128, L1], F32, name="psq", tag="ps_s")
                for dh in range(2):
                    nc.tensor.matmul(
                        psq[:],
                        wqT[dh][:, eh * 128 : (eh + 1) * 128],
                        xtT[dh][:],
                        start=(dh == 0),
                        stop=(dh == 1),
                    )
                nc.vector.tensor_copy(QT[eh][:], psq[:])
            # KTn[h] (128 e, 128 j) f32, negated (bias / scalar operand)
            KTn = [cpool.tile([128, JSH], F32, name=f"KTn{h}") for h in range(2)]
            for eh in range(2):
                psk = ps_pool.tile([128, JSH], F32, name="psk", tag="psk", bufs=1)
                for dh in range(2):
                    nc.tensor.matmul(
                        psk[:],
                        wkT[dh][:, eh * 128 : (eh + 1) * 128],
                        xssT[dh][:],
                        start=(dh == 0),
                        stop=(dh == 1),
                    )
                nc.vector.tensor_scalar(KTn[eh][:], psk[:], -1.0, None, AL.mult)


            # PE filler target (keeps HAM warm through the ramp; reuses the
            # prep psum slot, never read)
    
            wu2 = ps_pool.tile([32, 256], F32, name="wu2", tag="psk", bufs=1)

            def pe_fill(n):
                for _ in range(n):
                    nc.tensor.matmul(
                        wu2[:],
                        wz[:, 0:32],
                        wz[:, 0:256],
                        start=True,
                        stop=True,
                        skip_group_check=True,
                    )

            # ---- main loop: 32 quads of 4 source positions ----
            for g in range(NQUAD):
                ps = ps_pool.tile([128, L1], F32, name="ps", tag="ps_s")
                for jj in range(4):
                    j = 4 * g + jj
                    for h in range(2):
                        u = (g * 4 + jj) * 2 + h
                        eng = ENGINE_PATTERN[u % len(ENGINE_PATTERN)]
                        t = tpool.tile([128, L1], BF16, name="t", tag="t")
                        if eng == 1:
                            # t = relu(Q + (-K))
                            nc.scalar.activation(
                                t[:],
                                QT[h][:],
                                AF.Relu,
                                bias=KTn[h][:, j : j + 1],
                                scale=1.0,
                            )
                        else:
                            # t = max(Q + (-K), 0)
                            nc.vector.tensor_scalar(
                                t[:],
                                QT[h][:],
                                KTn[h][:, j : j + 1],
                                0.0,
                                AL.add,
                                AL.max,
                            )
                        nc.tensor.matmul(
                            ps[32 * jj : 32 * jj + 32, :],
                            sel[h][:],
                            t[:],
                            start=(h == 0),
                            stop=(h == 1),
                            tile_position=(0, 32 * jj),
                        )
                # p = exp(scores + b); pc = max(p, 1) = exp(relu(scores + b))
                p = wpool.tile([128, L1], BF16, name="p", tag="p", bufs=6)
                nc.scalar.activation(p[:], ps[:], AF.Exp, bias=bml[:], scale=1.0)
                pc = wpool.tile([128, L1], BF16, name="pc", tag="pc", bufs=16)
                nc.vector.tensor_scalar(pc[:], p[:], 1.0, None, AL.max)
                # accumulate partial outputs and denominators
                for eh in range(2):
                    off = (g % 2) * D + eh * 128
                    nc.tensor.matmul(
                        ops[eh][:],
                        V_sel[g // 2][:, off : off + 128],
                        pc[:],
                        start=(g == 0),
                        stop=(g == NQUAD - 1),
                        skip_group_check=True,
                    )
                nc.tensor.matmul(
                    sps[:],
                    ones_sel[:, 0:N],
                    pc[:],
                    start=(g == 0),
                    stop=(g == NQUAD - 1),
                    skip_group_check=True,
                )

            # ---- evacuate + store ----
            for eh in range(2):
                ou = wpool.tile([128, L1], F32, name="ou", tag="ou", bufs=2)
                ceng = nc.vector if eh == 0 else nc.scalar
                if eh == 0:
                    nc.vector.tensor_copy(ou[:], ops[eh][:])
                else:
                    nc.scalar.copy(ou[:], ops[eh][:])
                nc.sync.dma_start(outp_d[eh], ou[:])
            so = wpool.tile([16, L1], F32, name="so")
            nc.scalar.copy(so[:], sps[:])
            nc.sync.dma_start(souts_d[:], so[:])

    nc.compile()
    return nc


_CACHE: dict = {}


def _get_graph(b_val: float):
    key = round(float(b_val), 10)
    if key not in _CACHE:
        _CACHE[key] = _build(float(b_val))
    return _CACHE[key]


def _host_prep(x_source, x_target, Wq, Wk, Wv, w_mlp):
    """Build per-core input maps (numpy, bf16)."""
    w_full = np.tile(np.asarray(w_mlp, np.float32), D // G)  # w_full[d] = w[d%16]
    sel = np.zeros((2, 128, 32), np.float32)
    for h in range(2):
        for dl in range(128):
            d = 128 * h + dl
            sel[h, dl, d // G] = w_full[d]
    # V_sel mask: row p = 32*jj + s (s<16 valid), col e: keep if e%16 == s
    vmask = np.zeros((128, 2 * D), np.float32)
    for p in range(128):
        s = p % 32
        if s < 16:
            vmask[p, s::G] = 1.0
    # S selector: row p = 32*jj + s -> column s (s < 16)
    ones_sel = np.zeros((128, N), np.float32)
    for p in range(128):
        s = p % 32
        if s < 16:
            ones_sel[p, s] = 1.0

    def split_h(a):  # (256, X) -> (2, 128, X)
        return np.ascontiguousarray(a.reshape(2, 128, a.shape[1]))

    wq_b = split_h(np.asarray(Wq, np.float32).T).astype(BF)
    wk_b = split_h(np.asarray(Wk, np.float32).T).astype(BF)
    wv_b = split_h(np.asarray(Wv, np.float32).T).astype(BF)
    sel_b = sel.astype(BF)
    vmask_b = vmask.astype(BF)
    ones_b = ones_sel.astype(BF)

    xtT = [
        split_h(np.asarray(x_target[b], np.float32).T).astype(BF) for b in range(B)
    ]
    xsT = [np.asarray(x_source[b], np.float32).T for b in range(B)]
    in_maps = []
    for core in range(NCORES):
        b, jq = divmod(core, 4)
        j0 = jq * JSH
        xssT = split_h(xsT[b][:, j0 : j0 + JSH]).astype(BF)
        in_maps.append(
            {
                "xtT": xtT[b],
                "xssT": xssT,
                "wqT": wq_b,
                "wkT": wk_b,
                "wvT": wv_b,
                "sel": sel_b,
                "vmask": vmask_b,
                "ones_sel": ones_b,
            }
        )
    return in_maps


def _host_gather(results):
    """Sum partials over j-shards, normalize, reshape to (B, L1, D)."""
    out = np.empty((B, L1, D), np.float32)
    for b in range(B):
        cores = [b * 4 + jq for jq in range(4)]
        U = sum(
            results[c]["outp"].reshape(D, L1).astype(np.float64) for c in cores
        )  # (e, i)
        S = sum(results[c]["souts"].astype(np.float64) for c in cores)  # (nn, i)
        att = U / S[np.arange(D) % N, :]  # (e, i)
        out[b] = att.T.astype(np.float32)
    return out


def run(inputs, trace=False, **kwargs):
    nc = _get_graph(float(np.asarray(inputs["b_mlp"]).reshape(-1)[0]))
    in_maps = _host_prep(
        inputs["x_source"],
        inputs["x_target"],
        inputs["Wq"],
        inputs["Wk"],
        inputs["Wv"],
        inputs["w_mlp"],
    )
    res = run_bass_kernel_spmd(
        nc, in_maps, core_ids=list(range(NCORES)), trace=trace, **kwargs
    )
    return _host_gather(res.results), res


def kernel(**inputs) -> np.ndarray:
    out, _ = run(inputs, trace=False)
    return out


# revision 41
# speedup vs baseline: 1.1063x; 1.0391x over previous
"""Trainium2 Bass kernel for grouped vector attention (sparse_attention).

Reference computation (B=2, L1=L2=512, D=256, g=16, n=16):
    Q = x_target @ Wq.T ; K = x_source @ Wk.T ; V = x_source @ Wv.T
    diff = Q.reshape(B,L1,1,n,g) - K.reshape(B,1,L2,n,g)
    scores = relu(einsum('bijng,g->bijn', relu(diff), w_mlp) + b_mlp)
    att = softmax(scores, axis=2)                      # over L2
    out = einsum('bijn,bjgn->bign', att, V.reshape(B,L2,g,n)).reshape(B,L1,D)

Sharding: 8 cores = 2 batches x 4 L2(j)-quarters. Each core handles all 512
queries against its 128 source positions and produces partial (unnormalized)
outputs + partial softmax denominators; the host sums the 4 partials per
batch and divides. Sharding over j (not i) means the exp'd scores come out
with j on partitions — exactly what the att@V contraction needs, so there is
no on-chip transpose anywhere.

Per-core pipeline, for each quad of 4 source positions (32 quads):
  - tmp[d, i] = relu(Q[i,d] - K[j,d]) with d on partitions, i free:
      ScalarE:  activation(Relu, in=QT, bias=-K[:,j], scale=1)
      VectorE:  tensor_scalar(in=QT, s1=-K[:,j], s2=0, op0=add, op1=max)
  - grouped weighted sum over g=16 via TensorE matmul with block-diagonal
    [128 x 32] sel (w_mlp folded); j's 16 scores land in PSUM slot 32*jj.
  - p = exp(scores + b) off PSUM; pc = max(p, 1)   (= exp(relu(scores+b)))
  - V_sel[g][32*jj + nn, e] = V[4g+jj, e] * (e % 16 == nn)  (built once by a
    broadcast-DMA from a DRAM copy of V + one masked multiply)
  - out_partial[e, i]  += V_sel[g][:, e-half].T @ pc   (PSUM accumulation
    across all 32 quads);  S_partial[nn, i] += ones_sel.T @ pc
"""

import numpy as np

import concourse.bass as bass
import concourse.bacc as bacc
import concourse.tile as tile
import concourse.mybir as mybir
from concourse.bass_utils import run_bass_kernel_spmd

import ml_dtypes

F32 = mybir.dt.float32
BF16 = mybir.dt.bfloat16
AL = mybir.AluOpType
AF = mybir.ActivationFunctionType

B, L1, L2, D = 2, 512, 512, 256
G = 16           # group size (d_group)
N = 16           # number of groups
NCORES = 8
JSH = 128        # source positions per core (L2 / 4)
NQUAD = 32       # 32 quads of 4 source positions
BF = ml_dtypes.bfloat16

# elementwise engine rotation per (j,h) unit: 0=VectorE, 1=ScalarE, 2=GpSimd
# (GpSimd tensor_scalar measured 7.5us/op on HW - never assign it)
ENGINE_PATTERN = (
    0, 1, 0, 0, 1, 0, 1, 0, 0, 1, 0, 0, 1, 0, 1, 0,
    0, 1, 0, 0, 1, 0, 1, 0, 0, 1, 0, 0, 1, 0, 0, 0,
)
GPS_TT = False  # GpSimd streaming degrades VectorE via the shared SBUF port


def _build(b_val: float):
    """Build + compile the per-core Bass graph. Same graph for all 8 cores."""
    nc = bacc.Bacc(
        "TRN2", target_bir_lowering=False, debug=False, enable_asserts=False
    )

    # ---- DRAM parameters (per-core shards, host-prepped) ----
    xtT_d = nc.dram_tensor("xtT", [2, 128, L1], BF16, kind="ExternalInput")
    xssT_d = nc.dram_tensor("xssT", [2, 128, JSH], BF16, kind="ExternalInput")
    wqT_d = nc.dram_tensor("wqT", [2, 128, D], BF16, kind="ExternalInput")
    wkT_d = nc.dram_tensor("wkT", [2, 128, D], BF16, kind="ExternalInput")
    wvT_d = nc.dram_tensor("wvT", [2, 128, D], BF16, kind="ExternalInput")
    sel_d = nc.dram_tensor("sel", [2, 128, 32], BF16, kind="ExternalInput")
    vmask_d = nc.dram_tensor("vmask", [128, 2 * D], BF16, kind="ExternalInput")
    ones_d = nc.dram_tensor("ones_sel", [128, N], BF16, kind="ExternalInput")
    outp_d = nc.dram_tensor("outp", [2, 128, L1], F32, kind="ExternalOutput")
    souts_d = nc.dram_tensor("souts", [N, L1], F32, kind="ExternalOutput")
    vdram = nc.dram_tensor("vdram", [JSH, D], BF16)

    with tile.TileContext(nc) as tc:
        with (
            tc.tile_pool(name="const", bufs=1) as cpool,
            tc.tile_pool(name="vselp", bufs=1) as vpool,
            tc.tile_pool(name="work", bufs=4) as wpool,
            tc.tile_pool(name="tmps", bufs=9) as tpool,
            tc.tile_pool(name="ps_s", bufs=2, space="PSUM") as ps_pool,
            tc.tile_pool(name="ps_acc", bufs=1, space="PSUM") as pa_pool,
        ):
            # ---- load constants / inputs ----
            xtT = [cpool.tile([128, L1], BF16, name=f"xtT{h}") for h in range(2)]
            xssT = [cpool.tile([128, JSH], BF16, name=f"xssT{h}") for h in range(2)]
            wqT = [cpool.tile([128, D], BF16, name=f"wqT{h}") for h in range(2)]
            wkT = [cpool.tile([128, D], BF16, name=f"wkT{h}") for h in range(2)]
            wvT = [cpool.tile([128, D], BF16, name=f"wvT{h}") for h in range(2)]
            sel = [cpool.tile([128, 32], BF16, name=f"sel{h}") for h in range(2)]
            vmask = cpool.tile([128, 2 * D], BF16, name="vmask")
            ones_sel = cpool.tile([128, N], BF16, name="ones_sel")
            bml = cpool.tile([128, 1], F32, name="bml")
            nc.vector.memset(bml[:], float(b_val))

            # ---- accumulators (also the warm-up target: quad 0's V-matmul
            # uses start=True, which clears whatever the warm-up wrote) ----
            ops = [
                pa_pool.tile([128, L1], F32, name=f"ops{eh}") for eh in range(2)
            ]
            sps = pa_pool.tile([16, L1], F32, name="sps")

            # ---- PE warm-up burst: self-contained (memset inputs), runs at
            # t~0 so HAM flips to 8/8 and stays there until real matmuls flow
            wz = cpool.tile([128, L1], BF16, name="wz")
            nc.vector.memset(wz[:], 0.25)
            for k in range(20):
                nc.tensor.matmul(
                    ops[0][0:32, 0:256],
                    wz[:, 0:32],
                    wz[:, 0:256],
                    start=(k == 0),
                    stop=(k == 19),
                    skip_group_check=True,
                )
            for h in range(2):
                nc.sync.dma_start(xssT[h][:], xssT_d[h])
                nc.sync.dma_start(wvT[h][:], wvT_d[h])
                nc.sync.dma_start(sel[h][:], sel_d[h])
                nc.sync.dma_start(xtT[h][:], xtT_d[h])
                nc.scalar.dma_start(wqT[h][:], wqT_d[h])
                nc.scalar.dma_start(wkT[h][:], wkT_d[h])
            nc.scalar.dma_start(vmask[:], vmask_d[:])
            nc.scalar.dma_start(ones_sel[:], ones_d[:])

            # ---- V first: projection -> DRAM -> per-quad broadcast+mask ----
            Vt = cpool.tile([128, D], BF16, name="Vt")
            psv = ps_pool.tile([128, D], F32, name="psv", tag="psk", bufs=1)
            for dh in range(2):
                nc.tensor.matmul(
                    psv[:],
                    xssT[dh][:],
                    wvT[dh][:],
                    start=(dh == 0),
                    stop=(dh == 1),
                )
            nc.vector.tensor_copy(Vt[:], psv[:])
            nc.sync.dma_start(vdram[:], Vt[:])

            V_sel = []
            for k in range(NQUAD // 2):
                vs2 = vpool.tile([128, 2 * D], BF16, name=f"vs{k}")
                for half in range(2):
                    g = 2 * k + half
                    bsrc = (
                        vdram.ap()[4 * g : 4 * g + 4, :]
                        .unsqueeze(1)
                        .broadcast_to((4, 32, D))
                    )
                    nc.sync.dma_start(vs2[:, half * D : (half + 1) * D], bsrc)
                nc.vector.tensor_tensor(vs2[:], vs2[:], vmask[:], op=AL.mult)
                V_sel.append(vs2)


            # ---- projections on device ----
            # QT[h] (128 e, 512 i) bf16
            QT = [cpool.tile([128, L1], BF16, name=f"QT{h}") for h in range(2)]
            for eh in range(2):
                psq = ps_pool.tile([128, L1], F32, name="psq", tag="ps_s")
                for dh in range(2):
                    nc.tensor.matmul(
                        psq[:],
                        wqT[dh][:, eh * 128 : (eh + 1) * 128],
                        xtT[dh][:],
                        start=(dh == 0),
                        stop=(dh == 1),
                    )
                nc.vector.tensor_copy(QT[eh][:], psq[:])
            # KTn[h] (128 e, 128 j) f32, negated (bias / scalar operand)
            KTn = [cpool.tile([128, JSH], F32, name=f"KTn{h}") for h in range(2)]
            for eh in range(2):
                psk = ps_pool.tile([128, JSH], F32, name="psk", tag="psk", bufs=1)
                for dh in range(2):
                    nc.tensor.matmul(
                        psk[:],
                        wkT[dh][:, eh * 128 : (eh + 1) * 128],
                        xssT[dh][:],
                        start=(dh == 0),
                        stop=(dh == 1),
                    )
                nc.vector.tensor_scalar(KTn[eh][:], psk[:], -1.0, None, AL.mult)


            # PE filler target (keeps HAM warm through the ramp; reuses the
            # prep psum slot, never read)
    
            wu2 = ps_pool.tile([32, 256], F32, name="wu2", tag="psk", bufs=1)

            def pe_fill(n):
                for _ in range(n):
                    nc.tensor.matmul(
                        wu2[:],
                        wz[:, 0:32],
                        wz[:, 0:256],
                        start=True,
                        stop=True,
                        skip_group_check=True,
                    )

            # ---- main loop: 32 quads of 4 source positions ----
            for g in range(NQUAD):
                ps = ps_pool.tile([128, L1], F32, name="ps", tag="ps_s")
                for jj in range(4):
                    j = 4 * g + jj
                    for h in range(2):
                        u = (g * 4 + jj) * 2 + h
                        eng = ENGINE_PATTERN[u % len(ENGINE_PATTERN)]
                        t = tpool.tile([128, L1], BF16, name="t", tag="t")
                        if eng == 1:
                            # t = relu(Q + (-K))
                            nc.scalar.activation(
                                t[:],
                                QT[h][:],
                                AF.Relu,
                                bias=KTn[h][:, j : j + 1],
                                scale=1.0,
                            )
                        else:
                            # t = max(Q + (-K), 0)
                            nc.vector.tensor_scalar(
                                t[:],
                                QT[h][:],
                                KTn[h][:, j : j + 1],
                                0.0,
                                AL.add,
                                AL.max,
                            )
                        nc.tensor.matmul(
                            ps[32 * jj : 32 * jj + 32, :],
                            sel[h][:],
                            t[:],
                            start=(h == 0),
                            stop=(h == 1),
                            tile_position=(0, 32 * jj),
                        )
                # p = exp(scores + b); pc = max(p, 1) = exp(relu(scores + b))
                p = wpool.tile([128, L1], BF16, name="p", tag="p", bufs=6)
                nc.scalar.activation(p[:], ps[:], AF.Exp, bias=bml[:], scale=1.0)
                pc = wpool.tile([128, L1], BF16, name="pc", tag="pc", bufs=16)
                nc.vector.tensor_scalar(pc[:], p[:], 1.0, None, AL.max)
                # accumulate partial outputs and denominators
                for eh in range(2):
                    off = (g % 2) * D + eh * 128
                    nc.tensor.matmul(
                        ops[eh][:],
                        V_sel[g // 2][:, off : off + 128],
                        pc[:],
                        start=(g == 0),
                        stop=(g == NQUAD - 1),
                        skip_group_check=True,
                    )
                nc.tensor.matmul(
                    sps[:],
                    ones_sel[:, 0:N],
                    pc[:],
                    start=(g == 0),
                    stop=(g == NQUAD - 1),
                    skip_group_check=True,
                )

            # ---- evacuate + store ----
            for eh in range(2):
                ou = wpool.tile([128, L1], F32, name="ou", tag="ou", bufs=2)
                ceng = nc.vector if eh == 0 else nc.scalar
                if eh == 0:
                    nc.vector.tensor_copy(ou[:], ops[eh][:])
                else:
                    nc.scalar.copy(ou[:], ops[eh][:])
                nc.sync.dma_start(outp_d[eh], ou[:])
            so = wpool.tile([16, L1], F32, name="so")
            nc.scalar.copy(so[:], sps[:])
            nc.sync.dma_start(souts_d[:], so[:])

    nc.compile()
    return nc


_CACHE: dict = {}


def _get_graph(b_val: float):
    key = round(float(b_val), 10)
    if key not in _CACHE:
        _CACHE[key] = _build(float(b_val))
    return _CACHE[key]


def _host_prep(x_source, x_target, Wq, Wk, Wv, w_mlp):
    """Build per-core input maps (numpy, bf16)."""
    w_full = np.tile(np.asarray(w_mlp, np.float32), D // G)  # w_full[d] = w[d%16]
    sel = np.zeros((2, 128, 32), np.float32)
    for h in range(2):
        for dl in range(128):
            d = 128 * h + dl
            sel[h, dl, d // G] = w_full[d]
    # V_sel mask: row p = 32*jj + s (s<16 valid), col e: keep if e%16 == s
    vmask = np.zeros((128, 2 * D), np.float32)
    for p in range(128):
        s = p % 32
        if s < 16:
            vmask[p, s::G] = 1.0
    # S selector: row p = 32*jj + s -> column s (s < 16)
    ones_sel = np.zeros((128, N), np.float32)
    for p in range(128):
        s = p % 32
        if s < 16:
            ones_sel[p, s] = 1.0

    def split_h(a):  # (256, X) -> (2, 128, X)
        return np.ascontiguousarray(a.reshape(2, 128, a.shape[1]))

    wq_b = split_h(np.asarray(Wq, np.float32).T).astype(BF)
    wk_b = split_h(np.asarray(Wk, np.float32).T).astype(BF)
    wv_b = split_h(np.asarray(Wv, np.float32).T).astype(BF)
    sel_b = sel.astype(BF)
    vmask_b = vmask.astype(BF)
    ones_b = ones_sel.astype(BF)

    xtT = [
        split_h(np.asarray(x_target[b], np.float32).T).astype(BF) for b in range(B)
    ]
    xsT = [np.asarray(x_source[b], np.float32).T for b in range(B)]
    in_maps = []
    for core in range(NCORES):
        b, jq = divmod(core, 4)
        j0 = jq * JSH
        xssT = split_h(xsT[b][:, j0 : j0 + JSH]).astype(BF)
        in_maps.append(
            {
                "xtT": xtT[b],
                "xssT": xssT,
                "wqT": wq_b,
                "wkT": wk_b,
                "wvT": wv_b,
                "sel": sel_b,
                "vmask": vmask_b,
                "ones_sel": ones_b,
            }
        )
    return in_maps


def _host_gather(results):
    """Sum partials over j-shards, normalize, reshape to (B, L1, D)."""
    out = np.empty((B, L1, D), np.float32)
    for b in range(B):
        cores = [b * 4 + jq for jq in range(4)]
        U = sum(
            results[c]["outp"].reshape(D, L1).astype(np.float64) for c in cores
        )  # (e, i)
        S = sum(results[c]["souts"].astype(np.float64) for c in cores)  # (nn, i)
        att = U / S[np.arange(D) % N, :]  # (e, i)
        out[b] = att.T.astype(np.float32)
    return out


def run(inputs, trace=False, **kwargs):
    nc = _get_graph(float(np.asarray(inputs["b_mlp"]).reshape(-1)[0]))
    in_maps = _host_prep(
        inputs["x_source"],
        inputs["x_target"],
        inputs["Wq"],
        inputs["Wk"],
        inputs["Wv"],
        inputs["w_mlp"],
    )
    res = run_bass_kernel_spmd(
        nc, in_maps, core_ids=list(range(NCORES)), trace=trace, **kwargs
    )
    return _host_gather(res.results), res


def kernel(**inputs) -> np.ndarray:
    out, _ = run(inputs, trace=False)
    return out
